# revision 7
# baseline (speedup 1.0000x reference)
"""Trainium2 Bass kernel for the Crosscoder problem.

Model (reference):
    x   = concat(x_m, x_p)            [B=4096, 4096]
    pre = x @ W_enc.T + b_enc         [B, 16384]
    z   = topk64-mask(relu) of pre    [B, 16384]  (exactly the top-64 per row,
                                                   relu'd, scattered dense)
    rec_m = z @ W_dec_m.T             [B, 2048]
    rec_p = z @ W_dec_p.T             [B, 2048]
Returns (rec_m, rec_p, z).

Sharding: data-parallel over the 4096-token batch across 8 NeuronCores
(512 rows/core); encoder/decoder weights replicated per core.

Per-core device kernel (all fp32):
  Phase E: pre^(512x16384) tile-by-tile on PE (batch on PSUM partitions,
           hid chunks of 512 on free dim, contraction 33 k-tiles of 128 with
           the bias folded in as an extra contraction row). Each drained PSUM
           chunk is (a) DMA'd to a DRAM scratch `pre`, (b) scanned by DVE
           max8/match_replace (2 rounds -> per-chunk top-16 candidates).
  Phase C: exact top-64 of the 512 candidates per row -> threshold t
           (64th-largest). s = max(t, 0) since z keeps only positive values.
  Phase Z: z = (pre >= s) * pre, streamed from the DRAM scratch; z is
           written out and also PE-transposed into a DRAM zT scratch.
  Phase D: rec^T = W_decT.T @ zT on PE (contraction over hid, 128 k-tiles),
           out-dim groups of 1024 = 8 PSUM banks.
  Phase T: rec^T tiles PE-transposed back to rec row-major and written out.

Top-k exactness: per-chunk top-16 candidates are sufficient unless a single
512-wide chunk holds >16 of a row's global top-64 (P ~ 1e-10 per chunk, and
the miss only matters if it creates a spurious positive). Ties at the
threshold select all tied values. Both cases are detected on the host
(a z row with != 64 nonzeros when the 64th value is positive) and repaired
exactly on the CPU for the few affected rows (expected ~0.2 rows/run).
"""

import sys

for _p in ("/opt/trn_rl_repo", "/root/.axon_site/_ro/trn_rl_repo"):
    if _p not in sys.path:
        sys.path.insert(0, _p)

import numpy as np

import concourse.bacc as bacc
import concourse.mybir as mybir
import concourse.tile as tile
from concourse.bass_utils import run_bass_kernel_spmd
from concourse.masks import make_identity

f32 = mybir.dt.float32

# Problem shape (hardcoded per the harness contract)
B = 4096
D_IN = 2048
D_X = 2 * D_IN          # 4096 concat input
D_HID = 16384
K = 64
NCORES = 8
BC = B // NCORES        # 512 rows per core
P = 128

KT_E = D_X // P + 1     # 33 contraction tiles (last one carries the bias row)
D_XP = KT_E * P         # 4224 padded contraction
NCH = D_HID // 512      # 32 hid chunks of 512
MT = BC // P            # 4 batch sub-tiles per core
KT_D = D_HID // P       # 128 decode contraction tiles
NEG = -1e30


def build_kernel():
    nc = bacc.Bacc("TRN2", target_bir_lowering=False, debug=False)

    xT = nc.dram_tensor("xT", [D_XP, BC], f32, kind="ExternalInput")
    wE = nc.dram_tensor("wE", [D_XP, D_HID], f32, kind="ExternalInput")
    wDm = nc.dram_tensor("wDm", [D_HID, D_IN], f32, kind="ExternalInput")
    wDp = nc.dram_tensor("wDp", [D_HID, D_IN], f32, kind="ExternalInput")

    z_out = nc.dram_tensor("z", [BC, D_HID], f32, kind="ExternalOutput")
    rm_out = nc.dram_tensor("rec_m", [BC, D_IN], f32, kind="ExternalOutput")
    rp_out = nc.dram_tensor("rec_p", [BC, D_IN], f32, kind="ExternalOutput")

    pre_d = nc.dram_tensor("pre_scratch", [BC, D_HID], f32)
    zT_d = nc.dram_tensor("zT_scratch", [D_HID, BC], f32)

    with tile.TileContext(nc) as tc:
        # ---- persistent tiles ----
        import contextlib
        with contextlib.ExitStack() as outer:
            const = outer.enter_context(tc.tile_pool(name="const", bufs=1))
            ident = const.tile([P, P], f32)
            make_identity(nc, ident)
            cand = const.tile([P, MT * 512], f32, tag="cand")       # chunk top-16s
            ctop = const.tile([P, MT * 64], f32, tag="ctop")        # final top-64 values
            svec = const.tile([P, MT], f32, tag="svec")             # per-m threshold col

            # =======================  Phase E: encoder  =======================
            with tc.tile_pool(name="xsb", bufs=1) as xpool, \
                 tc.tile_pool(name="wenc", bufs=6) as wpool, \
                 tc.tile_pool(name="eps", bufs=2, space="PSUM") as epool:
                xsb = xpool.tile([P, KT_E * BC], f32, tag="xsb")
                for k in range(KT_E):
                    nc.sync.dma_start(out=xsb[:, k * BC:(k + 1) * BC],
                                      in_=xT[k * P:(k + 1) * P, :])
                for n in range(NCH):
                    pts = [epool.tile([P, 512], f32, tag=f"ps{m}", name=f"eps_{n}_{m}")
                           for m in range(MT)]
                    for k in range(KT_E):
                        wt = wpool.tile([P, 512], f32, tag="w")
                        nc.sync.dma_start(out=wt[:],
                                          in_=wE[k * P:(k + 1) * P, n * 512:(n + 1) * 512])
                        for m in range(MT):
                            nc.tensor.matmul(
                                pts[m][:],
                                xsb[:, k * BC + m * P: k * BC + (m + 1) * P],
                                wt[:],
                                start=(k == 0), stop=(k == KT_E - 1))
                    for m in range(MT):
                        stg = wpool.tile([P, 512], f32, tag="stg",
                                         name=f"stg_{n}_{m}")
                        nc.scalar.copy(stg[:], pts[m][:])
                        nc.sync.dma_start(
                            out=pre_d[m * P:(m + 1) * P, n * 512:(n + 1) * 512],
                            in_=stg[:])
                        # chunk top-16 candidates (destructive on the stage)
                        for r in range(2):
                            mm = cand[:, m * 512 + n * 16 + r * 8:
                                      m * 512 + n * 16 + (r + 1) * 8]
                            nc.vector.max(out=mm, in_=stg[:])
                            if r == 0:
                                nc.vector.match_replace(
                                    out=stg[:], in_to_replace=mm,
                                    in_values=stg[:], imm_value=NEG)

            # ==============  Phase C: top-64 of candidates -> t  ==============
            for m in range(MT):
                cm = cand[:, m * 512:(m + 1) * 512]
                for r in range(8):
                    c8 = ctop[:, m * 64 + r * 8: m * 64 + (r + 1) * 8]
                    nc.vector.max(out=c8, in_=cm)
                    if r < 7:
                        nc.vector.match_replace(out=cm, in_to_replace=c8,
                                                in_values=cm, imm_value=NEG)
                # s = max(t, 0) with t = 64th largest
                nc.vector.tensor_scalar_max(
                    svec[:, m:m + 1], ctop[:, m * 64 + 63: m * 64 + 64], 0.0)

            # ==================  Phase Z: z + zT transposes  ==================
            CH = 2048
            with tc.tile_pool(name="zio", bufs=3) as ziop, \
                 tc.tile_pool(name="zps", bufs=4, space="PSUM") as zpsp:
                for m in range(MT):
                    for j in range(D_HID // CH):
                        ld = ziop.tile([P, CH], f32, tag="ld")
                        nc.sync.dma_start(
                            out=ld[:], in_=pre_d[m * P:(m + 1) * P,
                                                 j * CH:(j + 1) * CH])
                        zt = ziop.tile([P, CH], f32, tag="zt")
                        nc.vector.scalar_tensor_tensor(
                            out=zt[:], in0=ld[:], scalar=svec[:, m:m + 1],
                            in1=ld[:], op0=mybir.AluOpType.is_ge,
                            op1=mybir.AluOpType.mult)
                        nc.sync.dma_start(
                            out=z_out[m * P:(m + 1) * P, j * CH:(j + 1) * CH],
                            in_=zt[:])
                        for t in range(CH // P):
                            tp = zpsp.tile([P, P], f32, tag="tp",
                                           name=f"ztp_{m}_{j}_{t}")
                            nc.tensor.transpose(
                                tp[:], zt[:, t * P:(t + 1) * P], ident[:])
                            ts = ziop.tile([P, P], f32, tag="ts",
                                           name=f"zts_{m}_{j}_{t}")
                            nc.scalar.copy(ts[:], tp[:])
                            row = j * CH + t * P
                            nc.sync.dma_start(
                                out=zT_d[row:row + P, m * P:(m + 1) * P],
                                in_=ts[:])

            # =======================  Phase D: decode  ========================
            rpool = outer.enter_context(tc.tile_pool(name="rsb", bufs=1))
            recT = rpool.tile([P, 2 * 2 * 8 * 512], f32, tag="recT")
            with tc.tile_pool(name="dio", bufs=6) as diop, \
                 tc.tile_pool(name="dps", bufs=1, space="PSUM") as dpool:
                for di, wD in enumerate((wDm, wDp)):
                    for og in range(2):
                        pts = [dpool.tile([P, 512], f32, tag=f"dp{mi}", name=f"dps_{di}_{og}_{mi}")
                               for mi in range(8)]
                        for k in range(KT_D):
                            zk = diop.tile([P, BC], f32, tag="zk")
                            nc.sync.dma_start(out=zk[:],
                                              in_=zT_d[k * P:(k + 1) * P, :])
                            wk = diop.tile([P, 1024], f32, tag="wd")
                            nc.sync.dma_start(
                                out=wk[:],
                                in_=wD[k * P:(k + 1) * P,
                                       og * 1024:(og + 1) * 1024])
                            for mi in range(8):
                                nc.tensor.matmul(
                                    pts[mi][:], wk[:, mi * P:(mi + 1) * P], zk[:],
                                    start=(k == 0), stop=(k == KT_D - 1))
                        for mi in range(8):
                            off = ((di * 2 + og) * 8 + mi) * 512
                            nc.scalar.copy(recT[:, off:off + 512], pts[mi][:])

            # ============  Phase T: transpose rec^T -> rec  ============
            with tc.tile_pool(name="tps", bufs=4, space="PSUM") as tpool, \
                 tc.tile_pool(name="tsb", bufs=4) as tsbp:
                for di, rout in enumerate((rm_out, rp_out)):
                    for og in range(2):
                        for mi in range(8):
                            off = ((di * 2 + og) * 8 + mi) * 512
                            for q in range(MT):
                                tp = tpool.tile([P, P], f32, tag="tq",
                                                name=f"rtp_{di}_{og}_{mi}_{q}")
                                nc.tensor.transpose(
                                    tp[:], recT[:, off + q * P: off + (q + 1) * P],
                                    ident[:])
                                ts = tsbp.tile([P, P], f32, tag="tr",
                                               name=f"rts_{di}_{og}_{mi}_{q}")
                                nc.scalar.copy(ts[:], tp[:])
                                col = og * 1024 + mi * P
                                nc.sync.dma_start(
                                    out=rout[q * P:(q + 1) * P, col:col + P],
                                    in_=ts[:])

    nc.compile()
    return nc


_NC_CACHE = None


def _get_nc():
    global _NC_CACHE
    if _NC_CACHE is None:
        _NC_CACHE = build_kernel()
    return _NC_CACHE


def _host_fix_rows(z, rec_m, rec_p, x, W_enc, b_enc, W_dec_m, W_dec_p, k):
    """Detect and exactly repair rows whose device top-k went wrong.

    A correct row has exactly min(64, #positive-top64) nonzeros; since the
    device mask keeps every value >= s (s = max(t64, 0)), an incorrect or
    tied row shows up as nnz > 64 (extra ties / missed candidates) only.
    """
    nnz = (z > 0).sum(axis=1)
    bad = np.nonzero(nnz > k)[0]
    for r in bad:
        pre = x[r].astype(np.float32) @ W_enc.T.astype(np.float32) + b_enc
        order = np.argsort(-pre, kind="stable")[:k]
        zr = np.zeros(D_HID, dtype=np.float32)
        zr[order] = np.maximum(pre[order], 0.0)
        z[r] = zr
        rec_m[r] = zr @ W_dec_m.T
        rec_p[r] = zr @ W_dec_p.T
    return len(bad)


def kernel(x_m, x_p, W_enc, b_enc, W_dec_m, W_dec_p, k):
    x_m = np.asarray(x_m, dtype=np.float32)
    x_p = np.asarray(x_p, dtype=np.float32)
    W_enc = np.asarray(W_enc, dtype=np.float32)
    b_enc = np.asarray(b_enc, dtype=np.float32)
    W_dec_m = np.asarray(W_dec_m, dtype=np.float32)
    W_dec_p = np.asarray(W_dec_p, dtype=np.float32)
    assert int(k) == K, f"kernel hardcodes k={K}, got {k}"

    # Host-side prep (cheap vs device transfer): transposed, padded operands.
    x = np.concatenate([x_m, x_p], axis=1)                   # [B, 4096]
    xT_full = np.empty((D_XP, B), dtype=np.float32)
    xT_full[:D_X] = x.T
    xT_full[D_X] = 1.0                                        # bias row
    xT_full[D_X + 1:] = 0.0
    wE = np.empty((D_XP, D_HID), dtype=np.float32)
    wE[:D_X] = W_enc.T
    wE[D_X] = b_enc
    wE[D_X + 1:] = 0.0
    wDm = np.ascontiguousarray(W_dec_m.T)                     # [16384, 2048]
    wDp = np.ascontiguousarray(W_dec_p.T)

    nc = _get_nc()
    in_maps = []
    for c in range(NCORES):
        in_maps.append({
            "xT": np.ascontiguousarray(xT_full[:, c * BC:(c + 1) * BC]),
            "wE": wE,
            "wDm": wDm,
            "wDp": wDp,
        })
    res = run_bass_kernel_spmd(nc, in_maps, list(range(NCORES)))

    z = np.empty((B, D_HID), dtype=np.float32)
    rec_m = np.empty((B, D_IN), dtype=np.float32)
    rec_p = np.empty((B, D_IN), dtype=np.float32)
    for c in range(NCORES):
        r = res.results[c]
        z[c * BC:(c + 1) * BC] = r["z"]
        rec_m[c * BC:(c + 1) * BC] = r["rec_m"]
        rec_p[c * BC:(c + 1) * BC] = r["rec_p"]

    _host_fix_rows(z, rec_m, rec_p, x, W_enc, b_enc, W_dec_m, W_dec_p, K)
    return rec_m, rec_p, z
